# revision 21
# baseline (speedup 1.0000x reference)
"""Burgers PDE RHS kernel for Trainium2 (8 NeuronCores, SPMD).

Reference math (per element i of the padded array U, DX = 0.01):
  delta = (l - 2c + r) / DX^2
  adv   = max(c,0)*(c-l)/DX + min(c,0)*(r-c)/DX
  out   = d*delta - adv,  d = sigmoid(d_org)*0.01

Algebraic reformulation (exact up to f32 rounding; verified vs reference):
  adv*DX = c*(r-c) - relu(c)*(l+r-2c)
  => out = (l+r-2c)*(relu(c)/DX + d/DX^2) - c*(r-c)/DX
In y-space (y = x/sqrt(DX) = 10x, host pre-scale):
  out = D*(relu(c_y) + beta) - c_y*(r_y - c_y),   D = l_y+r_y-2c_y,
  beta = d / DX^1.5 = d*1000.

Distribution: spatial dim sharded 8 ways; each core gets its S+2 element
slice (1-element halos resolved on the host from bc / neighboring shards).
On-chip layout is row-major [128 partitions x 32768 elems]; each DMA load
brings an overlapping [128, G+2] window so l/c/r are free-dim shifted views
of one SBUF tile.

Compute (2 DVE passes per element instead of the naive ~7):
  pass 1: t = l - c                      (stock tensor_tensor subtract)
  pass 2: one hand-written custom DVE op streaming s[j] = x[j+1]: the
          center value c is obtained as a 1-element temporal delay of the
          stream via the stage-0 swap flop (BYPASS latches operand B; the
          next element reads CURR_SWAP_OUT), so a single op computes
          w = r-c; D = w+t; out = D*(max(c,0)+beta) - c*w
          in one 8-stage pipeline pass. The first output element of each
          tile is garbage (stale swap flop) and is not stored.
"""

import os
import sys

import numpy as np

for _p in ("/opt/trn_rl_repo", "/root/.axon_site/_ro/trn_rl_repo"):
    if _p not in sys.path and os.path.isdir(_p):
        sys.path.append(_p)

import concourse.bacc as bacc
import concourse.bass as bass
import concourse.mybir as mybir
from concourse.ap import AP
from concourse.bass_utils import run_bass_kernel_spmd
from concourse.tile import TileContext

N_CORES = 8
N_TOTAL = 33554432
S = N_TOTAL // N_CORES          # 4194304 elements per core
P = 128
R = S // P                      # 32768 elements per partition row
DX = 0.01

# ---------------------------------------------------------------------------
# Custom DVE ops (registered once, idempotent)
# ---------------------------------------------------------------------------

def _register_ops():
    import concourse.dve_ops as dve_ops
    from concourse.dve_ops import DveOp, OPS, CUSTOM_DVE_SPECS, \
        _SUB_OPCODE_FOR_NAME, _CUSTOM_DVE_ROW_BASE, _COMPILE_CACHE
    from concourse.dve_spec import Spec, Src0, Src1, C0, C1, relu, lower, _has_src1
    from concourse.dve_uop import (
        AluInp, AluOp, DelayInp, DveOpSpec, InpSel, OutPath, OutSel, Trigger,
        UopConfig, ENABLE,
    )

    def _fused_uop():
        u = UopConfig()
        u.enable_input(InpSel.SRC_0, 0)     # r as blk0 ALU B operand
        u.enable_input(InpSel.SRC_0, 1)     # r on lane 0
        u.enable_input(InpSel.SRC_1, 2)     # t on lane 1
        u.enable_input(InpSel.ZERO, 3)      # 0 on lane 2 (for relu)
        u.enable_input(InpSel.CONST_0, 4)   # beta on lane 3
        u.require_inp0 = ENABLE
        u.require_inp1 = ENABLE
        u.trigger = (Trigger.SRC_TENSOR_DONE, Trigger.NONE, Trigger.NONE)
        u.next_uop = (0, 0, 0)
        u.enable_output(OutSel.ALU_OUT, OutPath.WR0_LO)

        b = u.datapath_config
        # blk0: temporal delay: out = swap (prev element = c); swap <- r.
        b[0].enable_alu(AluOp.BYPASS, AluInp.CURR_SWAP_OUT, AluInp.PREV_ALU_OUT)
        b[0].swap_enable = ENABLE
        b[0].pass_through_delay(0, 1, 2, 3)
        # blk1: w = r - c;  lane0 <- c.
        b[1].enable_alu(AluOp.SUBTRACT, AluInp.PREV_DELAY_0, AluInp.PREV_ALU_OUT)
        b[1].enable_delay_from_src(DelayInp.PREV_ALU_OUT, 0)
        b[1].pass_through_delay(1, 2, 3)
        # blk2: D = w + t;  lane1 <- w.
        b[2].enable_alu(AluOp.ADD, AluInp.PREV_ALU_OUT, AluInp.PREV_DELAY_1)
        b[2].enable_delay_from_src(DelayInp.PREV_ALU_OUT, 1)
        b[2].pass_through_delay(0, 2, 3)
        # blk3: P = max(c, 0);  lane2 <- D.
        b[3].enable_alu(AluOp.MAX, AluInp.PREV_DELAY_0, AluInp.PREV_DELAY_2)
        b[3].enable_delay_from_src(DelayInp.PREV_ALU_OUT, 2)
        b[3].pass_through_delay(0, 1, 3)
        # blk4: G = P + beta.
        b[4].enable_alu(AluOp.ADD, AluInp.PREV_ALU_OUT, AluInp.PREV_DELAY_3)
        b[4].pass_through_delay(0, 1, 2)
        # blk5: M = D * G.
        b[5].enable_alu(AluOp.MULTIPLY, AluInp.PREV_ALU_OUT, AluInp.PREV_DELAY_2)
        b[5].pass_through_delay(0, 1)
        # blk6: Z = c * w;  lane0 <- M.
        b[6].enable_alu(AluOp.MULTIPLY, AluInp.PREV_DELAY_0, AluInp.PREV_DELAY_1)
        b[6].enable_delay_from_src(DelayInp.PREV_ALU_OUT, 0)
        # blk7: out = M - Z.
        b[7].enable_alu(AluOp.SUBTRACT, AluInp.PREV_DELAY_0, AluInp.PREV_ALU_OUT)
        u.validate("v3")
        return u

    def _fused_reference(in0, in1, s0, s1, imm2):
        c = np.empty_like(in0)
        c[:, 1:] = in0[:, :-1]
        c[:, 0] = 0.0
        w = in0 - c
        D = w + in1
        return D * (np.maximum(c, 0) + s0) - c * w

    def _onepass_uop():
        """Single-input 8-stage op. Stream s[j] (host pre-scaled s = -y):
        two chained swap-flop delays recover c~ = s[j-1] and w~[j-1], so
        one pass computes the full stencil:
          c~ = s[j-1]; w~ = s[j]-s[j-1]; D = w~[j-1]-w~[j]  (= +D_y)
          G = beta - min(c~,0)  (= relu(c_y)+beta)
          out = G*D - c~*w~     (= the y-space output, correct sign)
        First TWO output elements per tile are garbage (stale swap flops)."""
        u = UopConfig()
        u.enable_input(InpSel.SRC_0, 0)     # s as blk0 ALU B operand
        u.enable_input(InpSel.SRC_0, 1)     # s on lane 0
        u.enable_input(InpSel.ZERO, 2)      # 0 on lane 1 (for min)
        u.enable_input(InpSel.CONST_0, 3)   # beta on lane 2
        u.require_inp0 = ENABLE
        u.trigger = (Trigger.SRC_TENSOR_DONE, Trigger.NONE, Trigger.NONE)
        u.next_uop = (0, 0, 0)
        u.enable_output(OutSel.ALU_OUT, OutPath.WR0_LO)

        b = u.datapath_config
        # blk0: temporal delay: out = swap (= s[j-1] = c~); swap <- s[j].
        b[0].enable_alu(AluOp.BYPASS, AluInp.CURR_SWAP_OUT, AluInp.PREV_ALU_OUT)
        b[0].swap_enable = ENABLE
        b[0].pass_through_delay(0, 1, 2)
        # blk1: w~ = s - c~;  lane0 <- c~.
        b[1].enable_alu(AluOp.SUBTRACT, AluInp.PREV_DELAY_0, AluInp.PREV_ALU_OUT)
        b[1].enable_delay_from_src(DelayInp.PREV_ALU_OUT, 0)
        b[1].pass_through_delay(1, 2)
        # blk2: D = w~[j-1] - w~[j]; swap <- w~ (SUBTRACT latches operand b);
        #       lane3 <- w~.
        b[2].enable_alu(AluOp.SUBTRACT, AluInp.CURR_SWAP_OUT, AluInp.PREV_ALU_OUT)
        b[2].swap_enable = ENABLE
        b[2].enable_delay_from_src(DelayInp.PREV_ALU_OUT, 3)
        b[2].pass_through_delay(0, 1, 2)
        # blk3: m = min(c~, 0);  lane4 <- D.
        b[3].enable_alu(AluOp.MIN, AluInp.PREV_DELAY_0, AluInp.PREV_DELAY_1)
        b[3].enable_delay_from_src(DelayInp.PREV_ALU_OUT, 4)
        b[3].pass_through_delay(0, 2, 3)
        # blk4: G = beta - m.
        b[4].enable_alu(AluOp.SUBTRACT, AluInp.PREV_DELAY_2, AluInp.PREV_ALU_OUT)
        b[4].pass_through_delay(0, 3, 4)
        # blk5: M = G * D.
        b[5].enable_alu(AluOp.MULTIPLY, AluInp.PREV_ALU_OUT, AluInp.PREV_DELAY_4)
        b[5].pass_through_delay(0, 3)
        # blk6: Z = c~ * w~;  lane5 <- M.
        b[6].enable_alu(AluOp.MULTIPLY, AluInp.PREV_DELAY_0, AluInp.PREV_DELAY_3)
        b[6].enable_delay_from_src(DelayInp.PREV_ALU_OUT, 5)
        # blk7: out = M - Z.
        b[7].enable_alu(AluOp.SUBTRACT, AluInp.PREV_DELAY_5, AluInp.PREV_ALU_OUT)
        u.validate("v3")
        return u

    def _onepass_reference(in0, in1, s0, s1, imm2):
        s = in0.astype(np.float32)
        c = np.zeros_like(s)
        c[:, 1:] = s[:, :-1]
        w = s - c
        wp = np.zeros_like(w)
        wp[:, 1:] = w[:, :-1]
        D = wp - w
        G = s0 - np.minimum(c, 0.0)
        return G * D - c * w

    class HandDveOp(DveOp):
        """DveOp whose table program is hand-written (bypasses lower())."""

        def __init__(self, name, fake_spec, uops, rd1_en=True):
            object.__setattr__(self, "name", name)
            object.__setattr__(self, "spec", fake_spec)
            object.__setattr__(self, "subdim", False)
            object.__setattr__(self, "uops_sha", {})
            object.__setattr__(self, "perf_en", {})
            object.__setattr__(self, "_uops", uops)
            object.__setattr__(self, "_rd1_en", rd1_en)

        def compile(self, ver):
            key = (self.name, ver)
            if (r := _COMPILE_CACHE.get(key)) is not None:
                return r
            from concourse.dve_ops import get_dve_sub_opcode
            result = DveOpSpec(
                name=self.name,
                opcode=get_dve_sub_opcode(self.name),
                uops=self._uops,
                rd1_en=self._rd1_en,
            )
            _COMPILE_CACHE[key] = result
            return result

    def _reg(op):
        if op.name in _SUB_OPCODE_FOR_NAME:
            return next(o for o in OPS if o.name == op.name)
        row = _CUSTOM_DVE_ROW_BASE + len(OPS)
        assert row < 0x20, "custom DVE row budget exceeded"
        OPS.append(op)
        _SUB_OPCODE_FOR_NAME[op.name] = row
        CUSTOM_DVE_SPECS[op.name] = op.spec
        return op

    fake = Spec(body=(Src0 + Src1) * (relu(Src0) + C0),
                reference=_fused_reference)
    fused = _reg(HandDveOp("BURGERS_FUSED_ANT", fake, [_fused_uop()]))

    fake1 = Spec(body=Src0 * (relu(Src0) + C0),
                 reference=_onepass_reference)
    onepass = _reg(HandDveOp("BURGERS_1P_ANT", fake1, [_onepass_uop()],
                             rd1_en=False))

    # fallback ops (3-pass x-space pipeline), kept for A/B testing
    def _reg_spec(name, spec):
        if name in _SUB_OPCODE_FOR_NAME:
            return next(o for o in OPS if o.name == name)
        row = _CUSTOM_DVE_ROW_BASE + len(OPS)
        assert row < 0x20
        sha = {}
        for ver in ("v3", "v4"):
            s = DveOpSpec(name=name, opcode=row, uops=lower(spec, ver=ver),
                          rd1_en=_has_src1(spec))
            sha[ver] = s.sha(ver)
        op = DveOp(name, spec, subdim=False, uops_sha=sha)
        OPS.append(op)
        _SUB_OPCODE_FOR_NAME[name] = row
        CUSTOM_DVE_SPECS[name] = spec
        return op

    w = Src1 - Src0
    spec_a = Spec(
        body=((w - Src0) * (relu(Src0) + C0) - Src0 * w) * C1,
        reference=lambda in0, in1, s0, s1, imm2:
            ((in1 - 2*in0) * (np.maximum(in0, 0) + s0) - in0 * (in1 - in0)) * s1,
    )
    spec_b = Spec(
        body=(Src0 * (relu(Src1) + C0)) * C1,
        reference=lambda in0, in1, s0, s1, imm2:
            (in0 * (np.maximum(in1, 0) + s0)) * s1,
    )
    op_a = _reg_spec("BURGERS_A_ANT", spec_a)
    op_b = _reg_spec("BURGERS_B_ANT", spec_b)
    return fused, onepass, op_a, op_b


OP_FUSED, OP_1P, OP_A, OP_B = _register_ops()

# ---------------------------------------------------------------------------
# Kernel build (cached)
# ---------------------------------------------------------------------------

_CACHE = {}


DEFAULT_SCHED = (2048, 4096, 4096, 4096, 4096, 4096, 4096, 4096, 2048)

# --- fused1c: engine-15 load rebalance via two-region layout ----------------
# HWDGE deals an n-partition copy to engines 0..d-1 (d = largest divisor of n
# <= 16) in consecutive chunks of n/d rows. SDMA engine 15 is ~17% slower per
# byte (HW-measured). Split the per-core array into region 1 = 128 rows x R1
# (dealt to all 16 engines) and region 2 = 120 rows x R2 (engines 0-14 only),
# so engine 15 gets R1/(R1+R2) = 0.853 of the per-engine bytes = its measured
# relative speed. 128*R1 + 120*R2 = S keeps the host layout unchanged.
R1 = 28208
R2 = 4864
assert 128 * R1 + 120 * R2 == S
SCHED1C = (  # (region, width); region-2 tiles interleaved, small tile last
    (1, 2048), (1, 3792), (1, 3792), (2, 2944), (1, 3792), (1, 3792),
    (1, 3792), (1, 3792), (1, 3408), (2, 1920),
)
assert sum(w for r, w in SCHED1C if r == 1) == R1
assert sum(w for r, w in SCHED1C if r == 2) == R2

# fused1d: fine-tuned balance (e15 ended 2.5us early under fused1c) + a
# [16]+[112] split of tile 0 (16-descriptor starter copy gets the engines
# moving ~1.5us sooner than waiting out 128 descriptors of generation).
R1D = 31298
R2D = 1568
assert 128 * R1D + 120 * R2D == S
SCHED1D = (
    (1, 2048), (1, 3664), (1, 3664), (1, 3664), (2, 800), (1, 3664),
    (1, 3664), (1, 3664), (1, 3664), (1, 3602), (2, 768),
)
assert sum(w for r, w in SCHED1D if r == 1) == R1D
assert sum(w for r, w in SCHED1D if r == 2) == R2D

# --- fused1b: per-port load balancing ---------------------------------------
# SDMA engine 15 (serving partitions 92-95 and 124-127 per the port swizzle
# port_id = ((p>>2)&7)<<1 | ((p>>6)&1)) is ~17% slower per byte than engines
# 0-14 (HW-measured, both f32 and fp16 runs). Give its 8 partitions shorter
# rows so all 16 engines finish together: 120*LA + 8*LB = S with
# LB/LA ~= 0.853 (the measured rate ratio).
LA = 33072
LB = 28208
assert 120 * LA + 8 * LB == S
# (row0, nrows, rowlen) partition segments in layout order; offsets cumulative.
SEGS = ((0, 92, LA), (92, 4, LB), (96, 28, LA), (124, 4, LB))
SCHED_A = (2048, 4128, 4128, 4128, 4128, 4128, 4128, 4128, 2128)
SCHED_B = (1744, 3520, 3520, 3520, 3520, 3520, 3520, 3520, 1824)
assert sum(SCHED_A) == LA and sum(SCHED_B) == LB


def build_nc(mode="fused1", tile_g=4096, x_bufs=4, io_bufs=4,
             sched=DEFAULT_SCHED, inplace=False, split_rings=True):
    key = (mode, tile_g, x_bufs, io_bufs, tuple(sched or ()), inplace,
           split_rings)
    if key in _CACHE:
        return _CACHE[key]
    if sched:
        widths = list(sched)
    else:
        widths = [tile_g] * (R // tile_g)
    assert sum(widths) == R, (sum(widths), R)
    f32 = mybir.dt.float32
    # fused1* runs the whole pipeline in fp16 I/O (DVE computes fp32
    # internally); halves HBM traffic, which is the bottleneck.
    dt_io = mybir.dt.float16 if mode.startswith("fused1") else f32

    nc = bacc.Bacc("TRN2", target_bir_lowering=False, debug=False)
    x = nc.dram_tensor("x", [S + 2], dt_io, kind="ExternalInput")
    k0 = nc.dram_tensor("k0", [P, 1], f32, kind="ExternalInput")
    y = nc.dram_tensor("y", [S], dt_io, kind="ExternalOutput")
    xh = getattr(x, "tensor", x)
    yh = getattr(y, "tensor", y)

    with TileContext(nc) as tc:
        with (
            tc.tile_pool(name="k", bufs=1) as kp,
            tc.tile_pool(name="x", bufs=x_bufs) as xp,
            tc.tile_pool(name="t", bufs=io_bufs) as tp,
            tc.tile_pool(name="o", bufs=io_bufs) as op_,
        ):
            # Loads go on the SP HWDGE ring; stores (and the tiny k0 load) on
            # the ACT HWDGE ring — separate FIFOs, so a store queued behind
            # the next tile's load can't head-of-line block it.
            st_eng = nc.scalar if split_rings else nc.sync
            k0t = kp.tile([P, 1], f32)
            st_eng.dma_start(out=k0t[:, :], in_=k0[:, :])
            if mode in ("fused1c", "fused1d"):
                r1, r2 = (R1, R2) if mode == "fused1c" else (R1D, R2D)
                sched = SCHED1C if mode == "fused1c" else SCHED1D
                base2 = 128 * r1
                tA = tB = 0
                first = mode == "fused1d"
                for region, G in sched:
                    if region == 1:
                        n, L, base, t = 128, r1, 0, tA
                        tA += G
                    else:
                        n, L, base, t = 120, r2, base2, tB
                        tB += G
                    xt = xp.tile([P, G + 2], dt_io, tag="x")
                    ot = op_.tile([P, G + 2], dt_io, tag="o")
                    if first:
                        # starter: 16-descriptor copy reaches the SDMA
                        # engines fast; the 112-row copy follows.
                        src = AP(xh, base + t, [[L, 16], [1, G + 2]])
                        nc.sync.dma_start(out=xt[0:16, 0:G + 2], in_=src)
                        src = AP(xh, base + t + 16 * L, [[L, 112], [1, G + 2]])
                        nc.sync.dma_start(out=xt[16:128, 0:G + 2], in_=src)
                        first = False
                    else:
                        src = AP(xh, base + t, [[L, n], [1, G + 2]])
                        nc.sync.dma_start(out=xt[0:n, 0:G + 2], in_=src)
                    nc.vector._custom_dve(OP_1P, out=ot[0:n, 0:G + 2],
                                          in0=xt[0:n, 0:G + 2],
                                          s0=k0t[0:n, :], s1=0.0)
                    dst = AP(yh, base + t, [[L, n], [1, G]])
                    st_eng.dma_start(out=dst, in_=ot[0:n, 2:G + 2])
                widths = []
            if mode == "fused1b":
                seg_off = []
                off = 0
                for row0, n, L in SEGS:
                    seg_off.append(off)
                    off += n * L
                assert off == S
                tA = tB = 0
                for GA, GB in zip(SCHED_A, SCHED_B):
                    xt = xp.tile([P, GA + 2], dt_io, tag="x")
                    ot = op_.tile([P, GA + 2], dt_io, tag="o")
                    for (row0, n, L), base in zip(SEGS, seg_off):
                        G, t = (GA, tA) if L == LA else (GB, tB)
                        src = AP(xh, base + t, [[L, n], [1, G + 2]])
                        nc.sync.dma_start(out=xt[row0:row0 + n, 0:G + 2],
                                          in_=src)
                    nc.vector._custom_dve(OP_1P, out=ot[:, 0:GA + 2],
                                          in0=xt[:, 0:GA + 2],
                                          s0=k0t[:, :], s1=0.0)
                    for (row0, n, L), base in zip(SEGS, seg_off):
                        G, t = (GA, tA) if L == LA else (GB, tB)
                        dst = AP(yh, base + t, [[L, n], [1, G]])
                        st_eng.dma_start(out=dst,
                                         in_=ot[row0:row0 + n, 2:G + 2])
                    tA += GA
                    tB += GB
                widths = []
            off = 0
            for G in widths:
                t0 = off
                off += G
                xt = xp.tile([P, G + 2], dt_io, tag="x")
                src = AP(xh, t0, [[R, P], [1, G + 2]])
                nc.sync.dma_start(out=xt[:, :], in_=src)
                dst = AP(yh, t0, [[R, P], [1, G]])
                if mode == "fused1":
                    ot = op_.tile([P, G + 2], dt_io, tag="o")
                    # out[j] valid for j>=2 (two stale swap flops per tile);
                    # centers t0+1..t0+G -> y[t0..t0+G).
                    nc.vector._custom_dve(OP_1P, out=ot[:, 0:G + 2],
                                          in0=xt[:, 0:G + 2],
                                          s0=k0t[:, :], s1=0.0)
                    st_eng.dma_start(out=dst, in_=ot[:, 2:G + 2])
                elif mode == "fused2":
                    tt = tp.tile([P, G + 1], f32, tag="t")
                    ot = tt if inplace else op_.tile([P, G + 1], f32, tag="o")
                    # t = l - c, aligned one col right (col 0 unused)
                    nc.vector.tensor_tensor(tt[:, 1:G + 1], xt[:, 0:G],
                                            xt[:, 1:G + 1],
                                            mybir.AluOpType.subtract)
                    nc.vector._custom_dve(OP_FUSED, out=ot[:, 0:G + 1],
                                          in0=xt[:, 1:G + 2],
                                          in1=tt[:, 0:G + 1],
                                          s0=k0t[:, :], s1=0.0)
                    st_eng.dma_start(out=dst, in_=ot[:, 1:G + 1])
                else:  # fused3: x-space 3-pass pipeline
                    at = tp.tile([P, G], f32, tag="t")
                    bt = op_.tile([P, G], f32, tag="o")
                    l = xt[:, 0:G]
                    c = xt[:, 1:G + 1]
                    r = xt[:, 2:G + 2]
                    nc.vector._custom_dve(OP_A, out=at[:, :], in0=c, in1=r,
                                          s0=k0t[:, :], s1=1.0 / DX)
                    nc.vector._custom_dve(OP_B, out=bt[:, :], in0=l, in1=c,
                                          s0=k0t[:, :], s1=1.0 / DX)
                    nc.vector.tensor_tensor(at[:, :], at[:, :], bt[:, :],
                                            mybir.AluOpType.add)
                    st_eng.dma_start(out=dst, in_=at[:, :])
    nc.compile()
    _CACHE[key] = nc
    return nc


# ---------------------------------------------------------------------------
# Host entry point
# ---------------------------------------------------------------------------

def _axon_device_reset():
    try:
        import ctypes
        import time as _time
        lib = ctypes.CDLL("/opt/axon/libaxon_pjrt.so")
        lib.axon_reset.restype = ctypes.c_int64
        lib.axon_reset()
        _time.sleep(2.0)
    except Exception:
        pass


def kernel(state, bc, d_org, _trace=False, _build_kwargs=None):
    state = np.asarray(state)
    bc = np.asarray(bc)
    d_org = np.asarray(d_org)
    in_dtype = state.dtype

    bk = dict(_build_kwargs or {})
    mode = bk.get("mode", "fused1")
    nc = build_nc(**bk)

    flat = state.reshape(-1).astype(np.float32, copy=False)
    bcf = bc.reshape(-1).astype(np.float32)
    d = np.float32(0.01) / (np.float32(1.0) + np.exp(-d_org.astype(np.float32)))

    if mode.startswith("fused1"):
        # s-space: pre-scale by -1/sqrt(DX) = -10 (the op's fixed SUBTRACT
        # operand order needs the negated stream to come out sign-correct);
        # beta = d/DX^1.5. fp16 end to end.
        U = np.empty(N_TOTAL + 2, dtype=np.float16)
        np.multiply(flat, np.float32(-10.0), out=U[1:-1], casting="unsafe")
        U[0] = np.float16(bcf[0] * np.float32(-10.0))
        U[-1] = np.float16(bcf[1] * np.float32(-10.0))
        k0_val = np.full((P, 1), np.float32(d) * np.float32(1000.0),
                         dtype=np.float32)
        in_maps = [
            {"x": U[c * S: c * S + S + 2], "k0": k0_val}
            for c in range(N_CORES)
        ]
        try:
            res = run_bass_kernel_spmd(nc, in_maps,
                                       core_ids=list(range(N_CORES)),
                                       trace=_trace)
        except Exception:
            _axon_device_reset()
            res = run_bass_kernel_spmd(nc, in_maps,
                                       core_ids=list(range(N_CORES)),
                                       trace=_trace)
        out = np.concatenate([res.results[c]["y"] for c in range(N_CORES)])
        out = out.reshape(1, 1, N_TOTAL).astype(in_dtype)
        if _trace:
            return out, res
        return out

    U = np.empty(N_TOTAL + 2, dtype=np.float32)
    if mode == "fused2":
        # y-space: pre-scale by 1/sqrt(DX) = 10; beta = d/DX^1.5
        np.multiply(flat, np.float32(10.0), out=U[1:-1])
        U[0] = bcf[0] * np.float32(10.0)
        U[-1] = bcf[1] * np.float32(10.0)
        k0_val = np.full((P, 1), np.float32(d) * np.float32(1000.0),
                         dtype=np.float32)
    else:
        U[0] = bcf[0]
        U[1:-1] = flat
        U[-1] = bcf[1]
        k0_val = np.full((P, 1), np.float32(d) * np.float32(1.0 / DX),
                         dtype=np.float32)

    in_maps = [
        {"x": U[c * S: c * S + S + 2], "k0": k0_val}
        for c in range(N_CORES)
    ]
    try:
        res = run_bass_kernel_spmd(nc, in_maps, core_ids=list(range(N_CORES)),
                                   trace=_trace)
    except Exception:
        # A prior crash can leave the accelerator wedged; reset and retry once.
        _axon_device_reset()
        res = run_bass_kernel_spmd(nc, in_maps, core_ids=list(range(N_CORES)),
                                   trace=_trace)
    out = np.concatenate([res.results[c]["y"] for c in range(N_CORES)])
    out = out.reshape(1, 1, N_TOTAL).astype(in_dtype, copy=False)
    if _trace:
        return out, res
    return out



# revision 24
# speedup vs baseline: 1.0191x; 1.0191x over previous
"""Burgers PDE RHS kernel for Trainium2 (8 NeuronCores, SPMD).

Reference math (per element i of the padded array U, DX = 0.01):
  delta = (l - 2c + r) / DX^2
  adv   = max(c,0)*(c-l)/DX + min(c,0)*(r-c)/DX
  out   = d*delta - adv,  d = sigmoid(d_org)*0.01

Algebraic reformulation (exact up to f32 rounding; verified vs reference):
  adv*DX = c*(r-c) - relu(c)*(l+r-2c)
  => out = (l+r-2c)*(relu(c)/DX + d/DX^2) - c*(r-c)/DX
In y-space (y = x/sqrt(DX) = 10x, host pre-scale):
  out = D*(relu(c_y) + beta) - c_y*(r_y - c_y),   D = l_y+r_y-2c_y,
  beta = d / DX^1.5 = d*1000.

Distribution: spatial dim sharded 8 ways; each core gets its S+2 element
slice (1-element halos resolved on the host from bc / neighboring shards).
On-chip layout is row-major [128 partitions x 32768 elems]; each DMA load
brings an overlapping [128, G+2] window so l/c/r are free-dim shifted views
of one SBUF tile.

Compute (2 DVE passes per element instead of the naive ~7):
  pass 1: t = l - c                      (stock tensor_tensor subtract)
  pass 2: one hand-written custom DVE op streaming s[j] = x[j+1]: the
          center value c is obtained as a 1-element temporal delay of the
          stream via the stage-0 swap flop (BYPASS latches operand B; the
          next element reads CURR_SWAP_OUT), so a single op computes
          w = r-c; D = w+t; out = D*(max(c,0)+beta) - c*w
          in one 8-stage pipeline pass. The first output element of each
          tile is garbage (stale swap flop) and is not stored.
"""

import os
import sys

import numpy as np

for _p in ("/opt/trn_rl_repo", "/root/.axon_site/_ro/trn_rl_repo"):
    if _p not in sys.path and os.path.isdir(_p):
        sys.path.append(_p)

import concourse.bacc as bacc
import concourse.bass as bass
import concourse.mybir as mybir
from concourse.ap import AP
from concourse.bass_utils import run_bass_kernel_spmd
from concourse.tile import TileContext

N_CORES = 8
N_TOTAL = 33554432
S = N_TOTAL // N_CORES          # 4194304 elements per core
P = 128
R = S // P                      # 32768 elements per partition row
DX = 0.01

# ---------------------------------------------------------------------------
# Custom DVE ops (registered once, idempotent)
# ---------------------------------------------------------------------------

def _register_ops():
    import concourse.dve_ops as dve_ops
    from concourse.dve_ops import DveOp, OPS, CUSTOM_DVE_SPECS, \
        _SUB_OPCODE_FOR_NAME, _CUSTOM_DVE_ROW_BASE, _COMPILE_CACHE
    from concourse.dve_spec import Spec, Src0, Src1, C0, C1, relu, lower, _has_src1
    from concourse.dve_uop import (
        AluInp, AluOp, DelayInp, DveOpSpec, InpSel, OutPath, OutSel, Trigger,
        UopConfig, ENABLE,
    )

    def _fused_uop():
        u = UopConfig()
        u.enable_input(InpSel.SRC_0, 0)     # r as blk0 ALU B operand
        u.enable_input(InpSel.SRC_0, 1)     # r on lane 0
        u.enable_input(InpSel.SRC_1, 2)     # t on lane 1
        u.enable_input(InpSel.ZERO, 3)      # 0 on lane 2 (for relu)
        u.enable_input(InpSel.CONST_0, 4)   # beta on lane 3
        u.require_inp0 = ENABLE
        u.require_inp1 = ENABLE
        u.trigger = (Trigger.SRC_TENSOR_DONE, Trigger.NONE, Trigger.NONE)
        u.next_uop = (0, 0, 0)
        u.enable_output(OutSel.ALU_OUT, OutPath.WR0_LO)

        b = u.datapath_config
        # blk0: temporal delay: out = swap (prev element = c); swap <- r.
        b[0].enable_alu(AluOp.BYPASS, AluInp.CURR_SWAP_OUT, AluInp.PREV_ALU_OUT)
        b[0].swap_enable = ENABLE
        b[0].pass_through_delay(0, 1, 2, 3)
        # blk1: w = r - c;  lane0 <- c.
        b[1].enable_alu(AluOp.SUBTRACT, AluInp.PREV_DELAY_0, AluInp.PREV_ALU_OUT)
        b[1].enable_delay_from_src(DelayInp.PREV_ALU_OUT, 0)
        b[1].pass_through_delay(1, 2, 3)
        # blk2: D = w + t;  lane1 <- w.
        b[2].enable_alu(AluOp.ADD, AluInp.PREV_ALU_OUT, AluInp.PREV_DELAY_1)
        b[2].enable_delay_from_src(DelayInp.PREV_ALU_OUT, 1)
        b[2].pass_through_delay(0, 2, 3)
        # blk3: P = max(c, 0);  lane2 <- D.
        b[3].enable_alu(AluOp.MAX, AluInp.PREV_DELAY_0, AluInp.PREV_DELAY_2)
        b[3].enable_delay_from_src(DelayInp.PREV_ALU_OUT, 2)
        b[3].pass_through_delay(0, 1, 3)
        # blk4: G = P + beta.
        b[4].enable_alu(AluOp.ADD, AluInp.PREV_ALU_OUT, AluInp.PREV_DELAY_3)
        b[4].pass_through_delay(0, 1, 2)
        # blk5: M = D * G.
        b[5].enable_alu(AluOp.MULTIPLY, AluInp.PREV_ALU_OUT, AluInp.PREV_DELAY_2)
        b[5].pass_through_delay(0, 1)
        # blk6: Z = c * w;  lane0 <- M.
        b[6].enable_alu(AluOp.MULTIPLY, AluInp.PREV_DELAY_0, AluInp.PREV_DELAY_1)
        b[6].enable_delay_from_src(DelayInp.PREV_ALU_OUT, 0)
        # blk7: out = M - Z.
        b[7].enable_alu(AluOp.SUBTRACT, AluInp.PREV_DELAY_0, AluInp.PREV_ALU_OUT)
        u.validate("v3")
        return u

    def _fused_reference(in0, in1, s0, s1, imm2):
        c = np.empty_like(in0)
        c[:, 1:] = in0[:, :-1]
        c[:, 0] = 0.0
        w = in0 - c
        D = w + in1
        return D * (np.maximum(c, 0) + s0) - c * w

    def _onepass_uop():
        """Single-input 8-stage op. Stream s[j] (host pre-scaled s = -y):
        two chained swap-flop delays recover c~ = s[j-1] and w~[j-1], so
        one pass computes the full stencil:
          c~ = s[j-1]; w~ = s[j]-s[j-1]; D = w~[j-1]-w~[j]  (= +D_y)
          G = beta - min(c~,0)  (= relu(c_y)+beta)
          out = G*D - c~*w~     (= the y-space output, correct sign)
        First TWO output elements per tile are garbage (stale swap flops)."""
        u = UopConfig()
        u.enable_input(InpSel.SRC_0, 0)     # s as blk0 ALU B operand
        u.enable_input(InpSel.SRC_0, 1)     # s on lane 0
        u.enable_input(InpSel.ZERO, 2)      # 0 on lane 1 (for min)
        u.enable_input(InpSel.CONST_0, 3)   # beta on lane 2
        u.require_inp0 = ENABLE
        u.trigger = (Trigger.SRC_TENSOR_DONE, Trigger.NONE, Trigger.NONE)
        u.next_uop = (0, 0, 0)
        u.enable_output(OutSel.ALU_OUT, OutPath.WR0_LO)

        b = u.datapath_config
        # blk0: temporal delay: out = swap (= s[j-1] = c~); swap <- s[j].
        b[0].enable_alu(AluOp.BYPASS, AluInp.CURR_SWAP_OUT, AluInp.PREV_ALU_OUT)
        b[0].swap_enable = ENABLE
        b[0].pass_through_delay(0, 1, 2)
        # blk1: w~ = s - c~;  lane0 <- c~.
        b[1].enable_alu(AluOp.SUBTRACT, AluInp.PREV_DELAY_0, AluInp.PREV_ALU_OUT)
        b[1].enable_delay_from_src(DelayInp.PREV_ALU_OUT, 0)
        b[1].pass_through_delay(1, 2)
        # blk2: D = w~[j-1] - w~[j]; swap <- w~ (SUBTRACT latches operand b);
        #       lane3 <- w~.
        b[2].enable_alu(AluOp.SUBTRACT, AluInp.CURR_SWAP_OUT, AluInp.PREV_ALU_OUT)
        b[2].swap_enable = ENABLE
        b[2].enable_delay_from_src(DelayInp.PREV_ALU_OUT, 3)
        b[2].pass_through_delay(0, 1, 2)
        # blk3: m = min(c~, 0);  lane4 <- D.
        b[3].enable_alu(AluOp.MIN, AluInp.PREV_DELAY_0, AluInp.PREV_DELAY_1)
        b[3].enable_delay_from_src(DelayInp.PREV_ALU_OUT, 4)
        b[3].pass_through_delay(0, 2, 3)
        # blk4: G = beta - m.
        b[4].enable_alu(AluOp.SUBTRACT, AluInp.PREV_DELAY_2, AluInp.PREV_ALU_OUT)
        b[4].pass_through_delay(0, 3, 4)
        # blk5: M = G * D.
        b[5].enable_alu(AluOp.MULTIPLY, AluInp.PREV_ALU_OUT, AluInp.PREV_DELAY_4)
        b[5].pass_through_delay(0, 3)
        # blk6: Z = c~ * w~;  lane5 <- M.
        b[6].enable_alu(AluOp.MULTIPLY, AluInp.PREV_DELAY_0, AluInp.PREV_DELAY_3)
        b[6].enable_delay_from_src(DelayInp.PREV_ALU_OUT, 5)
        # blk7: out = M - Z.
        b[7].enable_alu(AluOp.SUBTRACT, AluInp.PREV_DELAY_5, AluInp.PREV_ALU_OUT)
        u.validate("v3")
        return u

    def _onepass_reference(in0, in1, s0, s1, imm2):
        s = in0.astype(np.float32)
        c = np.zeros_like(s)
        c[:, 1:] = s[:, :-1]
        w = s - c
        wp = np.zeros_like(w)
        wp[:, 1:] = w[:, :-1]
        D = wp - w
        G = s0 - np.minimum(c, 0.0)
        return G * D - c * w

    class HandDveOp(DveOp):
        """DveOp whose table program is hand-written (bypasses lower())."""

        def __init__(self, name, fake_spec, uops, rd1_en=True):
            object.__setattr__(self, "name", name)
            object.__setattr__(self, "spec", fake_spec)
            object.__setattr__(self, "subdim", False)
            object.__setattr__(self, "uops_sha", {})
            object.__setattr__(self, "perf_en", {})
            object.__setattr__(self, "_uops", uops)
            object.__setattr__(self, "_rd1_en", rd1_en)

        def compile(self, ver):
            key = (self.name, ver)
            if (r := _COMPILE_CACHE.get(key)) is not None:
                return r
            from concourse.dve_ops import get_dve_sub_opcode
            result = DveOpSpec(
                name=self.name,
                opcode=get_dve_sub_opcode(self.name),
                uops=self._uops,
                rd1_en=self._rd1_en,
            )
            _COMPILE_CACHE[key] = result
            return result

    def _reg(op):
        if op.name in _SUB_OPCODE_FOR_NAME:
            return next(o for o in OPS if o.name == op.name)
        row = _CUSTOM_DVE_ROW_BASE + len(OPS)
        assert row < 0x20, "custom DVE row budget exceeded"
        OPS.append(op)
        _SUB_OPCODE_FOR_NAME[op.name] = row
        CUSTOM_DVE_SPECS[op.name] = op.spec
        return op

    fake = Spec(body=(Src0 + Src1) * (relu(Src0) + C0),
                reference=_fused_reference)
    fused = _reg(HandDveOp("BURGERS_FUSED_ANT", fake, [_fused_uop()]))

    fake1 = Spec(body=Src0 * (relu(Src0) + C0),
                 reference=_onepass_reference)
    onepass = _reg(HandDveOp("BURGERS_1P_ANT", fake1, [_onepass_uop()],
                             rd1_en=False))

    # fallback ops (3-pass x-space pipeline), kept for A/B testing
    def _reg_spec(name, spec):
        if name in _SUB_OPCODE_FOR_NAME:
            return next(o for o in OPS if o.name == name)
        row = _CUSTOM_DVE_ROW_BASE + len(OPS)
        assert row < 0x20
        sha = {}
        for ver in ("v3", "v4"):
            s = DveOpSpec(name=name, opcode=row, uops=lower(spec, ver=ver),
                          rd1_en=_has_src1(spec))
            sha[ver] = s.sha(ver)
        op = DveOp(name, spec, subdim=False, uops_sha=sha)
        OPS.append(op)
        _SUB_OPCODE_FOR_NAME[name] = row
        CUSTOM_DVE_SPECS[name] = spec
        return op

    w = Src1 - Src0
    spec_a = Spec(
        body=((w - Src0) * (relu(Src0) + C0) - Src0 * w) * C1,
        reference=lambda in0, in1, s0, s1, imm2:
            ((in1 - 2*in0) * (np.maximum(in0, 0) + s0) - in0 * (in1 - in0)) * s1,
    )
    spec_b = Spec(
        body=(Src0 * (relu(Src1) + C0)) * C1,
        reference=lambda in0, in1, s0, s1, imm2:
            (in0 * (np.maximum(in1, 0) + s0)) * s1,
    )
    op_a = _reg_spec("BURGERS_A_ANT", spec_a)
    op_b = _reg_spec("BURGERS_B_ANT", spec_b)
    return fused, onepass, op_a, op_b


OP_FUSED, OP_1P, OP_A, OP_B = _register_ops()

# ---------------------------------------------------------------------------
# Kernel build (cached)
# ---------------------------------------------------------------------------

_CACHE = {}


DEFAULT_SCHED = (2048, 4096, 4096, 4096, 4096, 4096, 4096, 4096, 2048)

# --- fused1c: engine-15 load rebalance via two-region layout ----------------
# HWDGE deals an n-partition copy to engines 0..d-1 (d = largest divisor of n
# <= 16) in consecutive chunks of n/d rows. SDMA engine 15 is ~17% slower per
# byte (HW-measured). Split the per-core array into region 1 = 128 rows x R1
# (dealt to all 16 engines) and region 2 = 120 rows x R2 (engines 0-14 only),
# so engine 15 gets R1/(R1+R2) = 0.853 of the per-engine bytes = its measured
# relative speed. 128*R1 + 120*R2 = S keeps the host layout unchanged.
R1 = 28208
R2 = 4864
assert 128 * R1 + 120 * R2 == S
SCHED1C = (  # (region, width); region-2 tiles interleaved, small tile last
    (1, 2048), (1, 3792), (1, 3792), (2, 2944), (1, 3792), (1, 3792),
    (1, 3792), (1, 3792), (1, 3408), (2, 1920),
)
assert sum(w for r, w in SCHED1C if r == 1) == R1
assert sum(w for r, w in SCHED1C if r == 2) == R2

# fused1d: fine-tuned balance (e15 ended 2.5us early under fused1c) + a
# [16]+[112] split of tile 0 (16-descriptor starter copy gets the engines
# moving ~1.5us sooner than waiting out 128 descriptors of generation).
# Result: 56.8us — WORSE than fused1c (extra tiles/copies cost more than the
# tweaks gained; the ramp is fixed latency, not descriptor-count).
R1D = 31298
R2D = 1568
assert 128 * R1D + 120 * R2D == S
SCHED1D = (
    (1, 2048), (1, 3664), (1, 3664), (1, 3664), (2, 800), (1, 3664),
    (1, 3664), (1, 3664), (1, 3664), (1, 3602), (2, 768),
)
assert sum(w for r, w in SCHED1D if r == 1) == R1D
assert sum(w for r, w in SCHED1D if r == 2) == R2D

# fused1e: fused1c's 10-tile structure, refined R1/R2 balance, small edge
# tiles (faster pipeline fill, shorter final-store tail), no starter split.
SCHED1E = (
    (1, 1024), (1, 4352), (1, 4352), (1, 4352), (2, 1024), (1, 4352),
    (1, 4352), (1, 4352), (1, 4162), (2, 544),
)
assert sum(w for r, w in SCHED1E if r == 1) == R1D
assert sum(w for r, w in SCHED1E if r == 2) == R2D

# --- fused1b: per-port load balancing ---------------------------------------
# SDMA engine 15 (serving partitions 92-95 and 124-127 per the port swizzle
# port_id = ((p>>2)&7)<<1 | ((p>>6)&1)) is ~17% slower per byte than engines
# 0-14 (HW-measured, both f32 and fp16 runs). Give its 8 partitions shorter
# rows so all 16 engines finish together: 120*LA + 8*LB = S with
# LB/LA ~= 0.853 (the measured rate ratio).
LA = 33072
LB = 28208
assert 120 * LA + 8 * LB == S
# (row0, nrows, rowlen) partition segments in layout order; offsets cumulative.
SEGS = ((0, 92, LA), (92, 4, LB), (96, 28, LA), (124, 4, LB))
SCHED_A = (2048, 4128, 4128, 4128, 4128, 4128, 4128, 4128, 2128)
SCHED_B = (1744, 3520, 3520, 3520, 3520, 3520, 3520, 3520, 1824)
assert sum(SCHED_A) == LA and sum(SCHED_B) == LB


def build_nc(mode="fused1", tile_g=4096, x_bufs=4, io_bufs=4,
             sched=DEFAULT_SCHED, inplace=False, split_rings=True,
             beta_imm=None):
    key = (mode, tile_g, x_bufs, io_bufs, tuple(sched or ()), inplace,
           split_rings, beta_imm)
    if key in _CACHE:
        return _CACHE[key]
    if sched:
        widths = list(sched)
    else:
        widths = [tile_g] * (R // tile_g)
    assert sum(widths) == R, (sum(widths), R)
    f32 = mybir.dt.float32
    # fused1* runs the whole pipeline in fp16 I/O (DVE computes fp32
    # internally); halves HBM traffic, which is the bottleneck.
    dt_io = mybir.dt.float16 if mode.startswith("fused1") else f32

    nc = bacc.Bacc("TRN2", target_bir_lowering=False, debug=False)
    x = nc.dram_tensor("x", [S + 2], dt_io, kind="ExternalInput")
    k0 = nc.dram_tensor("k0", [P, 1], f32, kind="ExternalInput")
    y = nc.dram_tensor("y", [S], dt_io, kind="ExternalOutput")
    xh = getattr(x, "tensor", x)
    yh = getattr(y, "tensor", y)

    with TileContext(nc) as tc:
        with (
            tc.tile_pool(name="k", bufs=1) as kp,
            tc.tile_pool(name="x", bufs=x_bufs) as xp,
            tc.tile_pool(name="t", bufs=io_bufs) as tp,
            tc.tile_pool(name="o", bufs=io_bufs) as op_,
        ):
            # Loads go on the SP HWDGE ring; stores (and the tiny k0 load) on
            # the ACT HWDGE ring — separate FIFOs, so a store queued behind
            # the next tile's load can't head-of-line block it.
            st_eng = nc.scalar if split_rings else nc.sync
            k0t = kp.tile([P, 1], f32)
            st_eng.dma_start(out=k0t[:, :], in_=k0[:, :])
            if mode in ("fused1c", "fused1d", "fused1e"):
                r1, r2 = (R1, R2) if mode == "fused1c" else (R1D, R2D)
                sched = {"fused1c": SCHED1C, "fused1d": SCHED1D,
                         "fused1e": SCHED1E}[mode]
                base2 = 128 * r1
                tA = tB = 0
                first = mode == "fused1d"
                for region, G in sched:
                    if region == 1:
                        n, L, base, t = 128, r1, 0, tA
                        tA += G
                    else:
                        n, L, base, t = 120, r2, base2, tB
                        tB += G
                    xt = xp.tile([P, G + 2], dt_io, tag="x")
                    ot = op_.tile([P, G + 2], dt_io, tag="o")
                    if first:
                        # starter: 16-descriptor copy reaches the SDMA
                        # engines fast; the 112-row copy follows.
                        src = AP(xh, base + t, [[L, 16], [1, G + 2]])
                        nc.sync.dma_start(out=xt[0:16, 0:G + 2], in_=src)
                        src = AP(xh, base + t + 16 * L, [[L, 112], [1, G + 2]])
                        nc.sync.dma_start(out=xt[16:128, 0:G + 2], in_=src)
                        first = False
                    else:
                        src = AP(xh, base + t, [[L, n], [1, G + 2]])
                        nc.sync.dma_start(out=xt[0:n, 0:G + 2], in_=src)
                    nc.vector._custom_dve(OP_1P, out=ot[0:n, 0:G + 2],
                                          in0=xt[0:n, 0:G + 2],
                                          s0=k0t[0:n, :], s1=0.0)
                    dst = AP(yh, base + t, [[L, n], [1, G]])
                    st_eng.dma_start(out=dst, in_=ot[0:n, 2:G + 2])
                widths = []
            if mode == "fused1b":
                seg_off = []
                off = 0
                for row0, n, L in SEGS:
                    seg_off.append(off)
                    off += n * L
                assert off == S
                tA = tB = 0
                for GA, GB in zip(SCHED_A, SCHED_B):
                    xt = xp.tile([P, GA + 2], dt_io, tag="x")
                    ot = op_.tile([P, GA + 2], dt_io, tag="o")
                    for (row0, n, L), base in zip(SEGS, seg_off):
                        G, t = (GA, tA) if L == LA else (GB, tB)
                        src = AP(xh, base + t, [[L, n], [1, G + 2]])
                        nc.sync.dma_start(out=xt[row0:row0 + n, 0:G + 2],
                                          in_=src)
                    nc.vector._custom_dve(OP_1P, out=ot[:, 0:GA + 2],
                                          in0=xt[:, 0:GA + 2],
                                          s0=k0t[:, :], s1=0.0)
                    for (row0, n, L), base in zip(SEGS, seg_off):
                        G, t = (GA, tA) if L == LA else (GB, tB)
                        dst = AP(yh, base + t, [[L, n], [1, G]])
                        st_eng.dma_start(out=dst,
                                         in_=ot[row0:row0 + n, 2:G + 2])
                    tA += GA
                    tB += GB
                widths = []
            off = 0
            for G in widths:
                t0 = off
                off += G
                xt = xp.tile([P, G + 2], dt_io, tag="x")
                src = AP(xh, t0, [[R, P], [1, G + 2]])
                nc.sync.dma_start(out=xt[:, :], in_=src)
                dst = AP(yh, t0, [[R, P], [1, G]])
                if mode == "fused1":
                    ot = op_.tile([P, G + 2], dt_io, tag="o")
                    # out[j] valid for j>=2 (two stale swap flops per tile);
                    # centers t0+1..t0+G -> y[t0..t0+G).
                    nc.vector._custom_dve(OP_1P, out=ot[:, 0:G + 2],
                                          in0=xt[:, 0:G + 2],
                                          s0=k0t[:, :], s1=0.0)
                    st_eng.dma_start(out=dst, in_=ot[:, 2:G + 2])
                elif mode == "fused2":
                    tt = tp.tile([P, G + 1], f32, tag="t")
                    ot = tt if inplace else op_.tile([P, G + 1], f32, tag="o")
                    # t = l - c, aligned one col right (col 0 unused)
                    nc.vector.tensor_tensor(tt[:, 1:G + 1], xt[:, 0:G],
                                            xt[:, 1:G + 1],
                                            mybir.AluOpType.subtract)
                    nc.vector._custom_dve(OP_FUSED, out=ot[:, 0:G + 1],
                                          in0=xt[:, 1:G + 2],
                                          in1=tt[:, 0:G + 1],
                                          s0=k0t[:, :], s1=0.0)
                    st_eng.dma_start(out=dst, in_=ot[:, 1:G + 1])
                else:  # fused3: x-space 3-pass pipeline
                    at = tp.tile([P, G], f32, tag="t")
                    bt = op_.tile([P, G], f32, tag="o")
                    l = xt[:, 0:G]
                    c = xt[:, 1:G + 1]
                    r = xt[:, 2:G + 2]
                    nc.vector._custom_dve(OP_A, out=at[:, :], in0=c, in1=r,
                                          s0=k0t[:, :], s1=1.0 / DX)
                    nc.vector._custom_dve(OP_B, out=bt[:, :], in0=l, in1=c,
                                          s0=k0t[:, :], s1=1.0 / DX)
                    nc.vector.tensor_tensor(at[:, :], at[:, :], bt[:, :],
                                            mybir.AluOpType.add)
                    st_eng.dma_start(out=dst, in_=at[:, :])
    nc.compile()
    _CACHE[key] = nc
    return nc


# ---------------------------------------------------------------------------
# Host entry point
# ---------------------------------------------------------------------------

def _axon_device_reset():
    try:
        import ctypes
        import time as _time
        lib = ctypes.CDLL("/opt/axon/libaxon_pjrt.so")
        lib.axon_reset.restype = ctypes.c_int64
        lib.axon_reset()
        _time.sleep(2.0)
    except Exception:
        pass


def kernel(state, bc, d_org, _trace=False, _build_kwargs=None):
    state = np.asarray(state)
    bc = np.asarray(bc)
    d_org = np.asarray(d_org)
    in_dtype = state.dtype

    bk = dict(_build_kwargs or {})
    mode = bk.get("mode", "fused1")
    nc = build_nc(**bk)

    flat = state.reshape(-1).astype(np.float32, copy=False)
    bcf = bc.reshape(-1).astype(np.float32)
    d = np.float32(0.01) / (np.float32(1.0) + np.exp(-d_org.astype(np.float32)))

    if mode.startswith("fused1"):
        # s-space: pre-scale by -1/sqrt(DX) = -10 (the op's fixed SUBTRACT
        # operand order needs the negated stream to come out sign-correct);
        # beta = d/DX^1.5. fp16 end to end.
        U = np.empty(N_TOTAL + 2, dtype=np.float16)
        np.multiply(flat, np.float32(-10.0), out=U[1:-1], casting="unsafe")
        U[0] = np.float16(bcf[0] * np.float32(-10.0))
        U[-1] = np.float16(bcf[1] * np.float32(-10.0))
        k0_val = np.full((P, 1), np.float32(d) * np.float32(1000.0),
                         dtype=np.float32)
        in_maps = [
            {"x": U[c * S: c * S + S + 2], "k0": k0_val}
            for c in range(N_CORES)
        ]
        try:
            res = run_bass_kernel_spmd(nc, in_maps,
                                       core_ids=list(range(N_CORES)),
                                       trace=_trace)
        except Exception:
            _axon_device_reset()
            res = run_bass_kernel_spmd(nc, in_maps,
                                       core_ids=list(range(N_CORES)),
                                       trace=_trace)
        out = np.concatenate([res.results[c]["y"] for c in range(N_CORES)])
        out = out.reshape(1, 1, N_TOTAL).astype(in_dtype)
        if _trace:
            return out, res
        return out

    U = np.empty(N_TOTAL + 2, dtype=np.float32)
    if mode == "fused2":
        # y-space: pre-scale by 1/sqrt(DX) = 10; beta = d/DX^1.5
        np.multiply(flat, np.float32(10.0), out=U[1:-1])
        U[0] = bcf[0] * np.float32(10.0)
        U[-1] = bcf[1] * np.float32(10.0)
        k0_val = np.full((P, 1), np.float32(d) * np.float32(1000.0),
                         dtype=np.float32)
    else:
        U[0] = bcf[0]
        U[1:-1] = flat
        U[-1] = bcf[1]
        k0_val = np.full((P, 1), np.float32(d) * np.float32(1.0 / DX),
                         dtype=np.float32)

    in_maps = [
        {"x": U[c * S: c * S + S + 2], "k0": k0_val}
        for c in range(N_CORES)
    ]
    try:
        res = run_bass_kernel_spmd(nc, in_maps, core_ids=list(range(N_CORES)),
                                   trace=_trace)
    except Exception:
        # A prior crash can leave the accelerator wedged; reset and retry once.
        _axon_device_reset()
        res = run_bass_kernel_spmd(nc, in_maps, core_ids=list(range(N_CORES)),
                                   trace=_trace)
    out = np.concatenate([res.results[c]["y"] for c in range(N_CORES)])
    out = out.reshape(1, 1, N_TOTAL).astype(in_dtype, copy=False)
    if _trace:
        return out, res
    return out

